# revision 40
# baseline (speedup 1.0000x reference)
"""Trainium2 Bass kernel for the DMIL/PCL detection loss (nms_detection).

Contract: kernel(cls_prob[500000,21] f32, boxes[500000,4] f32,
im_labels[1,20] i32) -> scalar f32 loss, matching the jax reference to
within fp32 tolerance.

Strategy (8 NeuronCores, SPMD), v2:
  - Shard the N=500000 proposal axis across 8 cores (62500 rows each,
    padded to 63488 = 128 partitions x 496 rows). Host reorders each
    shard class-major; only the NP present class planes + the background
    plane are shipped (absent classes never contribute).
  - Phase A (overlapped with the input DMA): per-class max via
    tensor_reduce; the winner's row position is extracted in ONE fused
    scalar_tensor_tensor (is_equal . iota, accumulated); winner box rows
    are pulled with a tiny PE matmul against the partition-winner mask;
    a DMA transpose roundtrip flips per-class data into [NP,*] rows;
    one AllGather exchanges (score, box) and every core selects the
    global winner per class.
  - Phase B: per-proposal max over present classes of
    log(inter) - log(area_b + area_g), which orders identically to IoU.
    Per class the geometry runs in 6 DVE instructions (2 tensor_scalar
    at 2x mode + 3 fused scalar_tensor_tensor/tensor_tensor + 1 sub),
    relu/ln on the scalar engine, the running max on GpSimd, with the
    class loop software-pipelined so no engine stalls on another.
  - Phase C: per-class counts / prob sums / weighted-log-bg sums via
    tensor_tensor_reduce (product + free-axis accumulate fused in one
    instruction) against threshold-masked copies of the row max; one
    TensorE column-sum matmul, one AllReduce, final scalar assembly.
"""

import os
import sys
from contextlib import ExitStack

import numpy as np

sys.path.insert(0, "/opt/trn_rl_repo")

NCORES = 8
N = 500000
C = 20
PERCORE = N // NCORES          # 62500
K = 496                        # rows per partition
ROWS = 128 * K                 # 63488 rows per core after padding
INV_N = 1.0 / N
LN13 = float(np.float32(np.log(1.0 / 3.0)))    # ov >= 0.5  <=>  z >= 1/3
LN111 = float(np.float32(np.log(1.0 / 11.0)))  # ov >= 0.1  <=>  z >= 1/11
TINY = 1e-30
BIG = 1e7
IOTA_PLANE = np.tile(np.arange(1, K + 1, dtype=np.float32), (128, 1))


def _build(present, dbg=False):
    import concourse.bacc as bacc
    import concourse.bass_isa as bass_isa
    import concourse.mybir as mybir
    from concourse import tile

    f32 = mybir.dt.float32
    Alu = mybir.AluOpType
    Act = mybir.ActivationFunctionType
    AX = mybir.AxisListType

    NP = len(present)

    nc = bacc.Bacc("TRN2", target_bir_lowering=False, debug=False,
                   num_devices=NCORES)
    # planes 0..NP-1 = present class probs (col cls+1), plane NP = background
    pin = nc.dram_tensor("p", [128, (NP + 1) * K], f32, kind="ExternalInput")
    bin_ = nc.dram_tensor("b", [128, 4 * K], f32, kind="ExternalInput")
    iin = nc.dram_tensor("iota", [128, K], f32, kind="ExternalInput")
    loss_out = nc.dram_tensor("loss", [1, 1], f32, kind="ExternalOutput")
    if dbg:
        dbg_g = nc.dram_tensor("dbg_g", [1, 6 * NP], f32, kind="ExternalOutput")
        dbg_f = nc.dram_tensor("dbg_f", [NP, 3], f32, kind="ExternalOutput")

    ctx = ExitStack()
    with ctx:
        tc = ctx.enter_context(tile.TileContext(nc))
        sb = ctx.enter_context(tc.tile_pool(name="sb", bufs=1))
        scr = ctx.enter_context(tc.tile_pool(name="scr", bufs=2))
        pipe = ctx.enter_context(tc.tile_pool(name="pipe", bufs=4))
        psum = ctx.enter_context(tc.tile_pool(name="psum", bufs=1, space="PSUM"))
        dram = ctx.enter_context(tc.tile_pool(name="dram", bufs=1, space="DRAM"))

        # ---------------- input loads --------------------------------------
        # iota first on the sync queue (first phase-A stt needs it); prob
        # planes stream on both queues; boxes last on the scalar queue
        # (first needed by the phase-A tail matmuls, ~35us in).
        IOTA1 = sb.tile([128, K], f32, tag="IOTA1")
        nc.sync.dma_start(out=IOTA1[:], in_=iin[:, :])
        P = []
        for q in range(NP + 1):
            t = sb.tile([128, K], f32, tag=f"P{q}")
            eng = nc.sync if q % 2 == 0 else nc.scalar
            eng.dma_start(out=t[:], in_=pin[:, q * K : (q + 1) * K])
            P.append(t)
        B = sb.tile([128, 4 * K], f32, tag="B")
        nc.scalar.dma_start(out=B[:], in_=bin_[:, :])
        Bx1 = B[:, 0 * K : 1 * K]
        By1 = B[:, 1 * K : 2 * K]
        Bx2 = B[:, 2 * K : 3 * K]
        By2 = B[:, 3 * K : 4 * K]

        # ---------------- phase A: per-class winner -------------------------
        ones128 = sb.tile([128, 1], f32, tag="ones128")
        nc.vector.memset(ones128[:], 1.0)

        M1 = sb.tile([128, NP], f32, tag="M1")
        POS = sb.tile([128, NP], f32, tag="POS")
        for q in range(NP):
            nc.vector.tensor_reduce(out=M1[:, q : q + 1], in_=P[q][:],
                                    axis=AX.X, op=Alu.max)
            j1 = scr.tile([128, K], f32, tag="jA")
            nc.vector.scalar_tensor_tensor(
                out=j1[:], in0=P[q][:], scalar=M1[:, q : q + 1], in1=IOTA1[:],
                op0=Alu.is_equal, op1=Alu.mult, accum_out=POS[:, q : q + 1])

        LM = sb.tile([128, NP], f32, tag="LM")
        nc.gpsimd.partition_all_reduce(LM[:], M1[:], channels=128,
                                       reduce_op=bass_isa.ReduceOp.max)
        PEQ = sb.tile([128, NP], f32, tag="PEQ")
        nc.vector.tensor_tensor(out=PEQ[:], in0=M1[:], in1=LM[:], op=Alu.is_equal)
        POSW = sb.tile([128, NP], f32, tag="POSW")
        nc.vector.tensor_tensor(out=POSW[:], in0=PEQ[:], in1=POS[:], op=Alu.mult)

        # winner k (column) per class: column-sum of POSW via PE
        KS1 = psum.tile([1, NP], f32, tag="KS1")
        nc.tensor.matmul(out=KS1[:], lhsT=ones128[:], rhs=POSW[:],
                         start=True, stop=True)
        KS1s = sb.tile([1, NP], f32, tag="KS1s")
        nc.scalar.copy(KS1s[:], KS1[:])
        # winner partition's box rows per class: PEQ-masked column sums
        BW = []
        for d in range(4):
            bw = psum.tile([NP, K], f32, tag=f"BW{d}")
            nc.tensor.matmul(out=bw[:], lhsT=PEQ[:],
                             rhs=B[:, d * K : (d + 1) * K], start=True, stop=True)
            BW.append(bw)

        # roundtrip (score, kpos) into [NP, 2] rows
        t1 = dram.tile([1, 2 * NP], f32)
        nc.sync.dma_start(out=t1[0:1, 0:NP], in_=LM[0:1, :])
        nc.sync.dma_start(out=t1[0:1, NP : 2 * NP], in_=KS1s[0:1, :])
        TP = sb.tile([NP, 2], f32, tag="TP")
        nc.sync.dma_start(out=TP[:].rearrange("p (d o) -> p d o", o=1),
                          in_=t1[0:1, :].rearrange("o (d p) -> p d o", d=2))

        E = sb.tile([NP, K], f32, tag="E")
        nc.vector.tensor_scalar(out=E[:], in0=IOTA1[0:NP, :], scalar1=TP[:, 1:2],
                                scalar2=None, op0=Alu.is_equal)
        CC = sb.tile([NP, 5], f32, tag="CC")
        nc.vector.tensor_copy(CC[:, 0:1], TP[:, 0:1])
        for d in range(4):
            je = scr.tile([NP, K], f32, tag="jE")
            nc.vector.scalar_tensor_tensor(
                out=je[:], in0=BW[d][:], scalar=1.0, in1=E[:],
                op0=Alu.mult, op1=Alu.mult, accum_out=CC[:, 1 + d : 2 + d])

        # ---------------- exchange winners across cores ---------------------
        ccin = dram.tile([NP, 5], f32)
        nc.sync.dma_start(out=ccin[:], in_=CC[:])
        ccout = dram.tile([NCORES, NP, 5], f32)
        nc.gpsimd.collective_compute(
            "AllGather", Alu.bypass,
            replica_groups=[list(range(NCORES))],
            ins=[ccin[:].opt()], outs=[ccout[:].opt()])

        # -------- prep work that fills the AllGather latency ----------------
        ONEK = sb.tile([128, 1], f32, tag="ONEK")
        nc.vector.memset(ONEK[:], 1.0)
        TINYT = sb.tile([128, 1], f32, tag="TINYT")
        nc.vector.memset(TINYT[:], TINY)
        bx2p = sb.tile([128, K], f32, tag="bx2p")
        nc.vector.tensor_scalar(out=bx2p[:], in0=Bx2, scalar1=1.0,
                                scalar2=None, op0=Alu.add)
        by2p = sb.tile([128, K], f32, tag="by2p")
        nc.vector.tensor_scalar(out=by2p[:], in0=By2, scalar1=1.0,
                                scalar2=None, op0=Alu.add)
        dbx = scr.tile([128, K], f32, tag="dbx")
        nc.gpsimd.tensor_sub(dbx[:], bx2p[:], Bx1)
        dby = scr.tile([128, K], f32, tag="dby")
        nc.gpsimd.tensor_sub(dby[:], by2p[:], By1)
        area_b = sb.tile([128, K], f32, tag="area_b")
        nc.vector.tensor_mul(area_b[:], dbx[:], dby[:])
        lp0 = sb.tile([128, K], f32, tag="lp0")
        nc.scalar.activation(lp0[:], P[NP][:], Act.Ln)
        RM = sb.tile([128, K], f32, tag="RM")

        # -------- select global winner, derive per-class constants ----------
        XG = sb.tile([NP, NCORES * 5], f32, tag="XG")
        nc.sync.dma_start(out=XG[:].rearrange("p (r d) -> p r d", d=5),
                          in_=ccout[:, :, :].rearrange("r p d -> p r d"))
        XGv = XG[:].rearrange("p (r d) -> p r d", d=5)

        gmax = sb.tile([NP, 1], f32, tag="gmax")
        nc.vector.tensor_reduce(out=gmax[:], in_=XGv[:, :, 0], axis=AX.X,
                                op=Alu.max)
        eq8 = sb.tile([NP, NCORES], f32, tag="eq8")
        nc.vector.tensor_scalar(out=eq8[:], in0=XGv[:, :, 0], scalar1=gmax[:],
                                scalar2=None, op0=Alu.is_equal)
        # winner-masked coords, all 4 at once: [NP, 4, 8] * eq8 bc
        XGc = XG[:].rearrange("p (r d) -> p d r", d=5)[:, 1:5, :]
        BBt = scr.tile([NP, 4 * NCORES], f32, tag="BBt")
        nc.vector.tensor_tensor(
            out=BBt[:].rearrange("p (d r) -> p d r", r=NCORES), in0=XGc,
            in1=eq8[:].rearrange("p (o r) -> p o r", o=1).broadcast_to(
                (NP, 4, NCORES)), op=Alu.mult)
        # T2 cols: gx1, gy1, gx2, gy2 (raw), gmax
        T2 = sb.tile([NP, 5], f32, tag="T2")
        nc.vector.tensor_reduce(out=T2[:, 0:4],
                                in_=BBt[:].rearrange("p (d r) -> p d r",
                                                     r=NCORES),
                                axis=AX.X, op=Alu.max)
        nc.vector.tensor_copy(T2[:, 4:5], gmax[:])

        # broadcast the [NP, 5] constants to all 128 partitions
        t2 = dram.tile([NP, 5], f32)
        nc.sync.dma_start(out=t2[:], in_=T2[:])
        RW = sb.tile([1, 5 * NP], f32, tag="RW")
        nc.sync.dma_start(out=RW[:].rearrange("o (d p) -> o d p", p=NP),
                          in_=t2[:, :].rearrange("(o p) d -> o d p", o=1))
        ones1 = sb.tile([1, 128], f32, tag="ones1")
        nc.vector.memset(ones1[:], 1.0)
        PS = psum.tile([128, 5 * NP], f32, tag="PS")
        nc.tensor.matmul(out=PS[:], lhsT=ones1[:], rhs=RW[:],
                         start=True, stop=True)
        GCON = sb.tile([128, 5 * NP], f32, tag="GCON")
        nc.scalar.copy(GCON[:], PS[:])
        # derived per-class rows: gx2p, gy2p, Ag
        GD = sb.tile([128, 3 * NP], f32, tag="GD")
        nc.vector.tensor_scalar(out=GD[:, 0:NP], in0=GCON[:, 2*NP:3*NP],
                                scalar1=1.0, scalar2=None, op0=Alu.add)
        nc.vector.tensor_scalar(out=GD[:, NP:2*NP], in0=GCON[:, 3*NP:4*NP],
                                scalar1=1.0, scalar2=None, op0=Alu.add)
        dgx = scr.tile([128, NP], f32, tag="dgx2")
        nc.vector.tensor_sub(dgx[:], GD[:, 0:NP], GCON[:, 0:NP])
        dgy = scr.tile([128, NP], f32, tag="dgy2")
        nc.vector.tensor_sub(dgy[:], GD[:, NP:2*NP], GCON[:, NP:2*NP])
        nc.vector.tensor_mul(GD[:, 2*NP:3*NP], dgx[:], dgy[:])
        if dbg:
            nc.sync.dma_start(out=dbg_g[:, 0:5*NP], in_=GCON[0:1, :])

        def gc(blk, j):           # [128,1] per-class constant column
            # blk: 0=gx1 1=gy1 2=gx2p 3=gy2p 4=Ag
            if blk in (0, 1):
                return GCON[:, blk * NP + j : blk * NP + j + 1]
            m = {2: 0, 3: 1, 4: 2}[blk]
            return GD[:, m * NP + j : m * NP + j + 1]

        # ---------------- phase B: z planes + running max -------------------
        ZL = sb.tile([128, NP * K], f32, tag="ZL")

        def zplane(j):
            return ZL[:, j * K : (j + 1) * K]

        def geom(j):
            ux = scr.tile([128, K], f32, tag="ux")
            nc.vector.tensor_scalar(out=ux[:], in0=Bx1, scalar1=gc(0, j),
                                    scalar2=None, op0=Alu.max)
            wx = scr.tile([128, K], f32, tag="wx")
            nc.vector.scalar_tensor_tensor(
                out=wx[:], in0=bx2p[:], scalar=gc(2, j), in1=ux[:],
                op0=Alu.min, op1=Alu.subtract)
            rx = scr.tile([128, K], f32, tag="rx")
            nc.scalar.activation(rx[:], wx[:], Act.Relu)
            uy = scr.tile([128, K], f32, tag="uy")
            nc.vector.tensor_scalar(out=uy[:], in0=By1, scalar1=gc(1, j),
                                    scalar2=None, op0=Alu.max)
            wy = scr.tile([128, K], f32, tag="wy")
            nc.vector.scalar_tensor_tensor(
                out=wy[:], in0=by2p[:], scalar=gc(3, j), in1=uy[:],
                op0=Alu.min, op1=Alu.subtract)
            qp = pipe.tile([128, K], f32, tag="qp")
            nc.gpsimd.tensor_mul(qp[:], rx[:], wy[:])
            la = pipe.tile([128, K], f32, tag="la")
            nc.scalar.activation(la[:], area_b[:], Act.Ln, bias=gc(4, j))
            return qp, la

        def finish(j, qp, la):
            inter = scr.tile([128, K], f32, tag="inter")
            nc.scalar.activation(inter[:], qp[:], Act.Relu)
            li = scr.tile([128, K], f32, tag="li")
            nc.scalar.activation(li[:], inter[:], Act.Ln, bias=TINYT[:])
            nc.gpsimd.tensor_sub(zplane(j), li[:], la[:])
            if j == 0:
                nc.vector.tensor_copy(RM[:], zplane(0))
            else:
                nc.vector.tensor_tensor(out=RM[:], in0=RM[:], in1=zplane(j),
                                        op=Alu.max)

        queue = [geom(jj) for jj in range(min(3, NP))]
        for j in range(NP):
            if j + 3 < NP:
                queue.append(geom(j + 3))
            finish(j, *queue[j])

        # ---------------- phase C: masked accumulations ---------------------
        ifgm = sb.tile([128, K], f32, tag="ifgm")
        nc.vector.tensor_scalar(out=ifgm[:], in0=RM[:], scalar1=LN13,
                                scalar2=None, op0=Alu.is_lt)
        RMF = sb.tile([128, K], f32, tag="RMF")
        nc.vector.scalar_tensor_tensor(
            out=RMF[:], in0=ifgm[:], scalar=-BIG, in1=RM[:],
            op0=Alu.mult, op1=Alu.add)
        nb1 = sb.tile([128, K], f32, tag="nb1")
        nc.vector.tensor_scalar(out=nb1[:], in0=RM[:], scalar1=LN111,
                                scalar2=None, op0=Alu.is_lt)
        RMBa = sb.tile([128, K], f32, tag="RMBa")
        nc.vector.scalar_tensor_tensor(
            out=RMBa[:], in0=nb1[:], scalar=-BIG, in1=RM[:],
            op0=Alu.mult, op1=Alu.add)
        fgm = sb.tile([128, K], f32, tag="fgm")
        nc.vector.tensor_scalar(out=fgm[:], in0=RM[:], scalar1=LN13,
                                scalar2=None, op0=Alu.is_ge)
        RMB = sb.tile([128, K], f32, tag="RMB")
        nc.vector.scalar_tensor_tensor(
            out=RMB[:], in0=fgm[:], scalar=-BIG, in1=RMBa[:],
            op0=Alu.mult, op1=Alu.add)

        ACCS = sb.tile([128, 2 * NP], f32, tag="ACCS")
        ACCN = sb.tile([128, NP], f32, tag="ACCN")
        for j in range(NP):
            eqb = pipe.tile([128, K], f32, tag="eqb")
            nc.vector.scalar_tensor_tensor(
                out=eqb[:], in0=zplane(j), scalar=1.0, in1=RMB[:],
                op0=Alu.mult, op1=Alu.is_equal)
            eqf = scr.tile([128, K], f32, tag="eqf")
            nc.vector.scalar_tensor_tensor(
                out=eqf[:], in0=zplane(j), scalar=1.0, in1=RMF[:],
                op0=Alu.mult, op1=Alu.is_equal,
                accum_out=ACCS[:, j : j + 1])
            jp = scr.tile([128, K], f32, tag="jp")
            nc.vector.scalar_tensor_tensor(
                out=jp[:], in0=eqf[:], scalar=1.0, in1=P[j][:],
                op0=Alu.mult, op1=Alu.mult,
                accum_out=ACCS[:, NP + j : NP + j + 1])
            jn = pipe.tile([128, K], f32, tag="jn")
            nc.gpsimd.tensor_mul(jn[:], eqb[:], lp0[:])
            jo = scr.tile([128, K], f32, tag="jo")
            nc.scalar.activation(jo[:], jn[:], Act.Copy,
                                 accum_out=ACCN[:, j : j + 1])

        nbase = 64 if 2 * NP > 32 else 32
        SUMB = psum.tile([nbase + 32, 1], f32, tag="SUMB")
        nc.tensor.matmul(out=SUMB[0 : 2 * NP, :], lhsT=ACCS[:], rhs=ones128[:],
                         start=True, stop=True)
        nc.tensor.matmul(out=SUMB[nbase : nbase + NP, :], lhsT=ACCN[:],
                         rhs=ones128[:], start=True, stop=True)

        SUMS = sb.tile([nbase + 32, 1], f32, tag="SUMS")
        nc.scalar.copy(SUMS[0 : 2 * NP, :], SUMB[0 : 2 * NP, :])
        nc.scalar.copy(SUMS[nbase : nbase + NP, :], SUMB[nbase : nbase + NP, :])

        # ---------------- AllReduce + final scalar assembly -----------------
        cc2in = dram.tile([3 * NP, 1], f32)
        nc.sync.dma_start(out=cc2in[0 : 2 * NP, :], in_=SUMS[0 : 2 * NP, :])
        nc.sync.dma_start(out=cc2in[2 * NP : 3 * NP, :],
                          in_=SUMS[nbase : nbase + NP, :])
        cc2out = dram.tile([3 * NP, 1], f32)
        nc.gpsimd.collective_compute(
            "AllReduce", Alu.add,
            replica_groups=[list(range(NCORES))],
            ins=[cc2in[:].opt()], outs=[cc2out[:].opt()])

        FIN = sb.tile([NP, 3], f32, tag="FIN")
        nc.sync.dma_start(out=FIN[:].rearrange("p (d o) -> p d o", o=1),
                          in_=cc2out[:, :].rearrange("(d p) o -> p d o", d=3))
        cntv = FIN[:, 0:1]
        spv = FIN[:, 1:2]
        ngv = FIN[:, 2:3]

        onesNP = sb.tile([NP, 1], f32, tag="onesNP")
        nc.vector.memset(onesNP[:], 1.0)
        mx = sb.tile([NP, 1], f32, tag="mx")
        nc.vector.tensor_scalar(out=mx[:], in0=cntv, scalar1=1.0,
                                scalar2=None, op0=Alu.max)
        rcv = sb.tile([NP, 1], f32, tag="rcv")
        nc.vector.reciprocal(rcv[:], mx[:])
        mean = sb.tile([NP, 1], f32, tag="mean")
        nc.vector.tensor_mul(mean[:], spv, rcv[:])
        icg = sb.tile([NP, 1], f32, tag="icg")
        nc.vector.tensor_scalar(out=icg[:], in0=cntv, scalar1=0.5,
                                scalar2=None, op0=Alu.is_lt)
        # lnm = ln(mean + [cnt==0]): 0 for empty classes, ln(mean) otherwise
        lnm = sb.tile([NP, 1], f32, tag="lnm")
        nc.scalar.activation(lnm[:], mean[:], Act.Ln, bias=icg[:])
        pv = sb.tile([NP, 1], f32, tag="pv")
        nc.vector.tensor_mul(pv[:], lnm[:], cntv)
        nc.vector.tensor_tensor(out=pv[:], in0=pv[:], in1=ngv, op=Alu.add)
        tot = sb.tile([NP, 1], f32, tag="tot")
        nc.vector.tensor_mul(tot[:], pv[:], T2[:, 4:5])

        LPS = psum.tile([1, 1], f32, tag="LPS")
        nc.tensor.matmul(out=LPS[:], lhsT=tot[:], rhs=onesNP[:],
                         start=True, stop=True)
        LS = sb.tile([1, 1], f32, tag="LS")
        nc.scalar.copy(LS[:], LPS[:])
        nc.scalar.mul(LS[:], LS[:], -INV_N)
        nc.sync.dma_start(out=loss_out[:, :], in_=LS[:])
        if dbg:
            nc.sync.dma_start(out=dbg_f[:, :], in_=FIN[:])

    nc.compile()
    return nc


def _shard_inputs(cls_prob, boxes, im_labels, present):
    cls_prob = np.ascontiguousarray(cls_prob, dtype=np.float32)
    boxes = np.ascontiguousarray(boxes, dtype=np.float32)
    NP = len(present)
    cols = [c + 1 for c in present] + [0]
    in_maps = []
    for core in range(NCORES):
        lo = core * PERCORE
        hi = lo + PERCORE
        p = np.zeros((ROWS, NP + 1), dtype=np.float32)
        p[:PERCORE] = cls_prob[lo:hi, cols]
        p[PERCORE:, NP] = 1.0                     # pad: ln(p0)=0, never argmax
        b = np.empty((ROWS, 4), dtype=np.float32)
        b[:PERCORE] = boxes[lo:hi]
        b[PERCORE:] = [-20000.0, -20000.0, -19999.0, -19999.0]   # zero-IoU pad
        # class-major / coord-major: [128, NP+1, 496] and [128, 4, 496]
        pcm = np.ascontiguousarray(
            p.reshape(128, K, NP + 1).transpose(0, 2, 1)).reshape(
                128, (NP + 1) * K)
        bcm = np.ascontiguousarray(
            b.reshape(128, K, 4).transpose(0, 2, 1)).reshape(128, 4 * K)
        in_maps.append({"p": pcm, "b": bcm, "iota": IOTA_PLANE})
    return in_maps


_CACHE = {}


def kernel(cls_prob, boxes, im_labels, _trace=False, _dbg=False):
    from concourse.bass_utils import run_bass_kernel_spmd

    present = tuple(int(c) for c in np.nonzero(np.asarray(im_labels)[0] > 0)[0])
    key = (present, _dbg)
    if key not in _CACHE:
        _CACHE[key] = _build(present, dbg=_dbg)
    nc = _CACHE[key]

    in_maps = _shard_inputs(cls_prob, boxes, im_labels, present)
    res = run_bass_kernel_spmd(nc, in_maps, list(range(NCORES)), trace=_trace)
    out = np.float32(res.results[0]["loss"][0, 0])
    if _trace or _dbg:
        kernel._last = res
    return np.asarray(out)


if __name__ == "__main__":
    cls_prob = np.load("/tmp/cls_prob.npy")
    boxes = np.load("/tmp/boxes.npy")
    im_labels = np.load("/tmp/im_labels.npy")
    dbg = os.environ.get("KDBG") == "1"
    out = kernel(cls_prob, boxes, im_labels, _dbg=dbg)
    print("kernel loss:", out)
    if dbg and hasattr(kernel, "_last"):
        r0 = kernel._last.results[0]
        for kk in ("dbg_g", "dbg_f"):
            if kk in r0:
                print(kk, np.array2string(r0[kk], precision=4,
                                          suppress_small=False))
